# revision 12
# baseline (speedup 1.0000x reference)
"""Fast-weight-sum (causal linear attention) transformer layer on 8 TRN2 cores.

Sharding: data-parallel over batch — BSZ=8 batches, one per NeuronCore, no
collectives. Each core runs the full layer for its batch column of h.

Per-core algorithm (L=1024, D=512, H=8 heads, dh=64, chunk C=128):
  qkv = h @ W_qkv in fp8(e4m3) with DoubleRow perf mode (K=256 per matmul;
        W_qkv columns pre-permuted on host to [Q|K|V] blocks)
  q <- elu(q)+1            (NOT normalized; EPS*sum_d(q) tracked via the
                            constant-1 column of s_ext)
  k <- (elu(k)+1) / sum_d  (normalized in the qkv phase, so v and the
                            attention state need no per-row scaling)
  Chunked causal linear attention, chunk-parallel:
    A^T[s,t] = k_s.q_t for 8 heads packed into 2 PSUM banks, masked with
    2 batched DVE multiplies per chunk.
    skv_c    = k_c^T @ [v|1|0] per head pair (chunk-local, all parallel)
    s_ext_c  = s_ext_{c-1} + skv_{c-1}   (tiny DVE adds; the only serial
               chain, ~150ns per link, 4 independent chains)
    out      = A^T.T @ [v|1|0]  +  q @ [S|kstate|1]   (per-head inter so
               cross-block garbage in s_ext is never contracted)
  denom = out[:,64] + EPS*out[:,65]; attn_h = (SCALE/denom_h) * out_h
          applied during the PSUM->SBUF copy (ACT per-partition scale)
  layer_out = attn @ W_o (bf16); out = layernorm(h + layer_out)

Input DMAs are split fine-grained and ordered by first use so the first
matmul starts ~2.5us in (vs ~19.5us with whole-tensor DMAs) and the PE's
HAM clock warms early.
"""

import numpy as np

import concourse.bass as bass
import concourse.mybir as mybir
import concourse.tile as tile
from concourse import bacc
from concourse.bass_utils import run_bass_kernel_spmd

L, D, F, H, DH = 1024, 512, 1536, 8, 64
P = 128
NLT = L // P          # 8 l-tiles == chunks
KD = D // P           # 4 contraction tiles of d_model
EW = DH + 2           # 66: per-head extended width [out | den | one]
EPS = 1e-5
LN_EPS = 1e-5
SCALE = 1.0 / np.sqrt(DH)
BF = mybir.dt.bfloat16
F8 = mybir.dt.float8e4
F32 = mybir.dt.float32
AX = mybir.AluOpType
ACTF = mybir.ActivationFunctionType
DR = mybir.MatmulPerfMode.DoubleRow
USE_FP8 = True

LAST_RESULT = None


def _build_core_kernel(nc, tc, apply_gb=True):
    # fp8 for the qkv matmul operands (host pre-cast + pre-shuffled so each
    # partition's DMA read is one contiguous run); bf16 elsewhere.
    QDT = F8 if USE_FP8 else BF
    hT_d = nc.dram_tensor("hT", (P, NLT, KD, P), QDT, kind="ExternalInput")
    wq_d = nc.dram_tensor("W_qkv", (P, KD, F), QDT, kind="ExternalInput")
    h_d = nc.dram_tensor("h", (P, NLT, D), BF, kind="ExternalInput")
    wo_d = nc.dram_tensor("W_o", (P, KD, D), BF, kind="ExternalInput")
    gamma_d = nc.dram_tensor("gamma", (D,), F32, kind="ExternalInput")
    beta_d = nc.dram_tensor("beta", (D,), F32, kind="ExternalInput")
    out_d = nc.dram_tensor("out", (L, D), F32, kind="ExternalOutput")

    with (
        tc.tile_pool(name="consts", bufs=1) as consts,
        tc.tile_pool(name="work", bufs=3) as work,
        tc.tile_pool(name="sext", bufs=3) as sext_pool,
        tc.tile_pool(name="pmm", bufs=2, space="PSUM") as pmm,
        tc.tile_pool(name="pscratch", bufs=4, space="PSUM") as pscratch,
        tc.tile_pool(name="ppb", bufs=2, space="PSUM") as ppb,
    ):
        # ---------- constants on gpsimd (ready before chunk 0) ----------
        # Causal mask replicated for 4 heads: utri4[s, j, t] = 1 iff s <= t.
        utri4 = consts.tile([P, 4, P], F32, tag="utri4")
        nc.gpsimd.memset(utri4, 0.0)
        nc.gpsimd.affine_select(
            out=utri4, in_=utri4, compare_op=AX.is_gt, fill=1.0,
            base=0, pattern=[[0, 4], [-1, P]], channel_multiplier=1,
        )
        # v_ext: per (lt, pair p) the 132 columns are [v_A |1|0| v_B |1|0];
        # only the v columns are filled per l-tile, the 1/0 columns are
        # constant (k is pre-normalized so no per-row scale is needed).
        v_ext = consts.tile([P, NLT, 4, 2 * EW], BF, tag="v_ext")
        vc = v_ext.rearrange("p l f (j e) -> p l f j e", e=EW)
        nc.gpsimd.memset(vc[:, :, :, :, DH:DH + 1], 1.0)
        nc.gpsimd.memset(vc[:, :, :, :, DH + 1:DH + 2], 0.0)
        # s_ext for chunk 0: zeros except the per-head constant-1 column
        # (feeds sum_d(q) into out[:,65] through the inter matmul).
        s0 = consts.tile([P, 4, 2 * EW], BF, tag="s0")
        nc.gpsimd.memset(s0, 0.0)
        s0c = s0.rearrange("p f (j e) -> p f j e", e=EW)
        nc.gpsimd.memset(s0c[0:DH, :, 0:1, DH + 1:DH + 2], 1.0)
        nc.gpsimd.memset(s0c[DH:P, :, 1:2, DH + 1:DH + 2], 1.0)

        eps_sb = consts.tile([P, 1], F32, tag="eps_sb")
        nc.vector.memset(eps_sb, LN_EPS)

        # ---------- input DMAs: fine-grained, ordered by first use ----------
        # scalar (HWDGE) ring: the qkv operands, first-needed first.
        wq_b = consts.tile([P, KD, F], QDT, tag="wq_b")
        hT = consts.tile([P, NLT, KD, P], QDT, tag="hT")
        nc.scalar.dma_start(wq_b[:, 0:2, 0:D], wq_d[:, 0:2, 0:D])
        nc.scalar.dma_start(hT[:, 0], hT_d[:, 0])
        nc.scalar.dma_start(wq_b[:, 2:4, 0:D], wq_d[:, 2:4, 0:D])
        nc.scalar.dma_start(wq_b[:, 0:2, D:2 * D], wq_d[:, 0:2, D:2 * D])
        nc.scalar.dma_start(wq_b[:, 2:4, D:2 * D], wq_d[:, 2:4, D:2 * D])
        nc.scalar.dma_start(wq_b[:, 0:2, 2 * D:F], wq_d[:, 0:2, 2 * D:F])
        nc.scalar.dma_start(wq_b[:, 2:4, 2 * D:F], wq_d[:, 2:4, 2 * D:F])
        for lt in range(1, NLT):
            nc.scalar.dma_start(hT[:, lt], hT_d[:, lt])
        # gpsimd (SWDGE) ring: late consumers (out-proj and residual).
        h_bf = consts.tile([P, NLT, D], BF, tag="h_bf")
        wo_b = consts.tile([P, KD, D], BF, tag="wo_b")
        nc.gpsimd.dma_start(wo_b, wo_d[:])
        nc.gpsimd.dma_start(h_bf[:, 0:4], h_d[:, 0:4])
        nc.gpsimd.dma_start(h_bf[:, 4:8], h_d[:, 4:8])
        if apply_gb:
            gamma_ap = gamma_d[:]
            gamma_bc = consts.tile([P, D], BF, tag="gamma_bc")
            nc.gpsimd.dma_start(
                gamma_bc,
                bass.AP(tensor=gamma_ap.tensor, offset=gamma_ap.offset,
                        ap=[[0, P]] + list(gamma_ap.ap)),
            )
            beta_ap = beta_d[:]
            beta_bc = consts.tile([P, D], F32, tag="beta_bc")
            nc.gpsimd.dma_start(
                beta_bc,
                bass.AP(tensor=beta_ap.tensor, offset=beta_ap.offset,
                        ap=[[0, P]] + list(beta_ap.ap)),
            )

        # ---------- qkv projection + feature map ----------
        # q and k share one tensor so a single XBAR transpose per l-tile
        # covers both: qk_sb[:, lt, 0:512] = q, [:, lt, 512:1024] = k-hat.
        qk_sb = consts.tile([P, NLT, 2 * D], BF, tag="qk_sb")
        for lt in range(NLT):
            for g in range(3):  # 0=q, 1=k, 2=v
                pm = pmm.tile([P, D], F32, tag="mm")
                if USE_FP8:
                    for k2 in range(2):
                        nc.tensor.matmul(
                            pm,
                            lhsT=hT[:, lt, 2 * k2:2 * k2 + 2, :],
                            rhs=wq_b[:, 2 * k2:2 * k2 + 2, g * D:(g + 1) * D],
                            start=(k2 == 0),
                            stop=(k2 == 1),
                            perf_mode=DR,
                        )
                else:
                    for kt in range(KD):
                        nc.tensor.matmul(
                            pm,
                            lhsT=hT[:, lt, kt, :],
                            rhs=wq_b[:, kt, g * D:(g + 1) * D],
                            start=(kt == 0),
                            stop=(kt == KD - 1),
                        )
                if g == 2:
                    nc.vector.tensor_copy(
                        vc[:, lt, :, :, 0:DH],
                        pm.rearrange("p (f j e) -> p f j e", f=4, j=2),
                    )
                else:
                    # elu(x)+1 == min(exp(x), 1) + relu(x)
                    e1 = work.tile([P, D], BF, tag="fmap_e")
                    nc.scalar.activation(e1, pm, ACTF.Exp)
                    r1 = work.tile([P, D], BF, tag="fmap_r")
                    nc.scalar.activation(r1, pm, ACTF.Relu)
                    if g == 0:
                        nc.vector.scalar_tensor_tensor(
                            out=qk_sb[:, lt, 0:D], in0=e1, scalar=1.0,
                            in1=r1, op0=AX.min, op1=AX.add)
                    else:
                        kt = work.tile([P, D], BF, tag="fmap_k")
                        nc.vector.scalar_tensor_tensor(
                            out=kt, in0=e1, scalar=1.0,
                            in1=r1, op0=AX.min, op1=AX.add)
                        ksum = work.tile([P, H], F32, tag="ksum")
                        nc.vector.reduce_sum(
                            out=ksum,
                            in_=kt.rearrange("p (h e) -> p h e", e=DH),
                            axis=mybir.AxisListType.X,
                        )
                        krec = work.tile([P, H], F32, tag="krec")
                        nc.vector.reciprocal(krec, ksum)
                        nc.vector.tensor_tensor(
                            qk_sb[:, lt, D:2 * D].rearrange(
                                "p (h e) -> p h e", e=DH),
                            kt.rearrange("p (h e) -> p h e", e=DH),
                            krec[:, :, None].to_broadcast((P, H, DH)),
                            AX.mult,
                        )

        # ---------- q^T, k^T via one XBAR DMA transpose per l-tile ----------
        qkT = consts.tile([P, NLT, 8, P], BF, tag="qkT")
        for lt in range(NLT):
            nc.sync.dma_start_transpose(qkT[:, lt], qk_sb[:, lt])

        # ---------- chunked causal linear attention ----------
        attn = consts.tile([P, NLT, D], BF, tag="attn")
        s_cur = s0
        for c in range(NLT):
            # A^T for 8 heads into 2 banks: bank X rows<-head A (contraction
            # partitions 0:64), bank Y head B (64:128) — concurrent in PE.
            abX = pscratch.tile([P, 4, P], F32, tag="scr", name="abX")
            abY = pscratch.tile([P, 4, P], F32, tag="scr", name="abY")
            for p in range(4):
                nc.tensor.matmul(abX[:, p], lhsT=qkT[0:DH, c, 4 + p, :],
                                 rhs=qkT[0:DH, c, p, :], start=True, stop=True)
                nc.tensor.matmul(abY[:, p], lhsT=qkT[DH:P, c, 4 + p, :],
                                 rhs=qkT[DH:P, c, p, :], start=True, stop=True)
            # batched causal masks: one DVE multiply per bank
            amX = work.tile([P, 4, P], BF, tag="amX")
            amY = work.tile([P, 4, P], BF, tag="amY")
            nc.vector.tensor_tensor(amX, abX, utri4, AX.mult)
            nc.vector.tensor_tensor(amY, abY, utri4, AX.mult)

            # chunk-local state: skv[f, :] = sum_s khat[s,f] * [v|1|0]
            # (cross-head blocks are garbage; per-head inter never reads them)
            skv = [pscratch.tile([P, 2, 2 * EW], F32, tag="scr",
                                 name=f"skv{i}") for i in range(2)]
            for p in range(4):
                nc.tensor.matmul(
                    skv[p // 2][:, p % 2],
                    lhsT=qk_sb[:, c, D + p * P:D + (p + 1) * P],
                    rhs=v_ext[:, c, p], start=True, stop=True)

            # intra + inter into pb; per-head inter keeps the contraction
            # inside the valid diagonal blocks of s_cur.
            pbs = [ppb.tile([P, 2, 2 * EW], F32, tag="pb", name=f"pb{i}")
                   for i in range(2)]
            # all 8 matmuls into one bank form a single accumulation group
            # (start clears the bank; per-element has_written makes the first
            # write to each column range an overwrite, the second an add)
            for i in range(2):
                for j in range(2):
                    p = 2 * i + j
                    pb = pbs[i][:, j]
                    nc.tensor.matmul(pb[:, 0:EW], lhsT=qkT[0:DH, c, p, :],
                                     rhs=s_cur[0:DH, p, 0:EW],
                                     start=(j == 0), stop=False)
                    nc.tensor.matmul(pb[:, 0:EW], lhsT=amX[:, p],
                                     rhs=v_ext[:, c, p, 0:EW],
                                     start=False, stop=False)
                    nc.tensor.matmul(pb[:, EW:2 * EW], lhsT=qkT[DH:P, c, p, :],
                                     rhs=s_cur[DH:P, p, EW:2 * EW],
                                     start=False, stop=False)
                    nc.tensor.matmul(pb[:, EW:2 * EW], lhsT=amY[:, p],
                                     rhs=v_ext[:, c, p, EW:2 * EW],
                                     start=False, stop=(j == 1))

            # prefix state for the next chunk: s_next = s_cur + skv
            if c < NLT - 1:
                s_next = sext_pool.tile([P, 4, 2 * EW], BF, tag="s_ext")
                for p in range(4):
                    nc.vector.tensor_tensor(s_next[:, p], s_cur[:, p],
                                            skv[p // 2][:, p % 2], AX.add)
                s_cur = s_next

            # denominators + scaled copy-out (ACT per-partition scale)
            dq = work.tile([P, 2, 2, 2, 2], F32, tag="dq")
            den = work.tile([P, H], F32, tag="den")
            denr = work.tile([P, H], F32, tag="denr")
            for i in range(2):
                pbr = pbs[i].rearrange("p f (j e) -> p f j e", e=EW)
                nc.scalar.copy(dq[:, i], pbr[:, :, :, DH:DH + 2])
            nc.vector.scalar_tensor_tensor(
                out=den.rearrange("p (i f j) -> p i f j", i=2, f=2),
                in0=dq[:, :, :, :, 1], scalar=EPS,
                in1=dq[:, :, :, :, 0], op0=AX.mult, op1=AX.add)
            nc.vector.reciprocal(denr, den)
            nc.vector.tensor_scalar_mul(denr, denr, SCALE)
            ac = attn[:, c].rearrange("p (f e) -> p f e", e=DH)
            for i in range(2):
                pbr = pbs[i].rearrange("p f (j e) -> p f j e", e=EW)
                nc.vector.tensor_tensor(
                    ac[:, 4 * i:4 * i + 4, :],
                    pbr[:, :, :, 0:DH],
                    denr[:, 4 * i:4 * i + 4, None].to_broadcast((P, 4, DH)),
                    AX.mult,
                )

        # ---------- attn^T via XBAR DMA transpose ----------
        attnT = consts.tile([P, NLT, KD, P], BF, tag="attnT")
        for c in range(NLT):
            nc.sync.dma_start_transpose(attnT[:, c], attn[:, c])

        # ---------- output projection + residual + layernorm ----------
        for lt in range(NLT):
            pm = pmm.tile([P, D], F32, tag="mm")
            for kt in range(KD):
                nc.tensor.matmul(pm, lhsT=attnT[:, lt, kt, :],
                                 rhs=wo_b[:, kt], start=(kt == 0),
                                 stop=(kt == KD - 1))
            x = work.tile([P, D], F32, tag="lnx")
            nc.vector.tensor_add(out=x, in0=pm, in1=h_bf[:, lt])
            stats = work.tile([P, nc.vector.BN_STATS_DIM], F32, tag="stats")
            nc.vector.bn_stats(out=stats, in_=x)
            mv = work.tile([P, nc.vector.BN_AGGR_DIM], F32, tag="mv")
            nc.vector.bn_aggr(out=mv, in_=stats)
            std = work.tile([P, 1], F32, tag="std")
            nc.scalar.activation(std, mv[:, 1:2], ACTF.Sqrt, bias=eps_sb,
                                 scale=1.0)
            rstd = work.tile([P, 1], F32, tag="rstd")
            nc.vector.reciprocal(rstd, std)
            # nmr = -mean * rstd, so ACT can apply (x - mean)*rstd as
            # Identity(x*rstd + nmr) with per-partition scale/bias.
            nmr = work.tile([P, 1], F32, tag="nmr")
            nc.vector.tensor_scalar(out=nmr, in0=mv[:, 0:1], scalar1=-1.0,
                                    scalar2=rstd, op0=AX.mult, op1=AX.mult)
            xn = work.tile([P, D], F32 if not apply_gb else BF, tag="xn")
            nc.scalar.activation(xn, x, ACTF.Identity, bias=nmr, scale=rstd)
            if apply_gb:
                xg = work.tile([P, D], BF, tag="xg")
                nc.vector.tensor_tensor(xg, xn, gamma_bc, AX.mult)
                yo = work.tile([P, D], F32, tag="yo")
                nc.vector.tensor_tensor(yo, xg, beta_bc, AX.add)
                nc.scalar.dma_start(out_d[lt * P:(lt + 1) * P, :], yo)
            else:
                nc.scalar.dma_start(out_d[lt * P:(lt + 1) * P, :], xn)


_NC_CACHE = {}


def _get_nc(apply_gb=True):
    key = ("nc", apply_gb)
    if key not in _NC_CACHE:
        nc = bacc.Bacc("TRN2", target_bir_lowering=False, debug=False)
        with tile.TileContext(nc) as tc:
            _build_core_kernel(nc, tc, apply_gb=apply_gb)
        nc.compile()
        _NC_CACHE[key] = nc
    return _NC_CACHE[key]


def kernel(h, W_qkv, W_o, gamma, beta, trace=False):
    global LAST_RESULT
    h = np.asarray(h, dtype=np.float32)
    W_qkv = np.asarray(W_qkv, dtype=np.float32)
    W_o = np.asarray(W_o, dtype=np.float32)
    gamma = np.asarray(gamma, dtype=np.float32)
    beta = np.asarray(beta, dtype=np.float32)

    import ml_dtypes
    bf16 = ml_dtypes.bfloat16
    f8 = ml_dtypes.float8_e4m3fn if USE_FP8 else bf16
    # Permute W_qkv columns from per-head [q|k|v] interleave to [Q|K|V]
    # blocks, convert to fp8, and lay out partition-major so each SBUF
    # partition's DMA read is one contiguous run.
    w_perm = np.ascontiguousarray(
        W_qkv.reshape(D, H, 3, DH).transpose(0, 2, 1, 3).reshape(D, F)
        .reshape(KD, P, F).transpose(1, 0, 2)).astype(f8)
    wo_shuf = np.ascontiguousarray(
        W_o.reshape(KD, P, D).transpose(1, 0, 2)).astype(bf16)

    # gamma==1, beta==0 (the spec's fill) makes the gamma/beta ops exact
    # no-ops; specialize the kernel to skip them in that case.
    apply_gb = not (np.all(gamma == 1.0) and np.all(beta == 0.0))
    nc = _get_nc(apply_gb)
    in_maps = []
    for b in range(8):
        hb = h[:, b, :]
        in_maps.append({
            "h": np.ascontiguousarray(
                hb.reshape(NLT, P, D).transpose(1, 0, 2)).astype(bf16),
            "hT": np.ascontiguousarray(
                hb.reshape(NLT, P, KD, P).transpose(3, 0, 2, 1)).astype(f8),
            "W_qkv": w_perm,
            "W_o": wo_shuf,
            "gamma": gamma,
            "beta": beta,
        })
    res = run_bass_kernel_spmd(nc, in_maps, core_ids=list(range(8)), trace=trace)
    LAST_RESULT = res
    return np.stack([res.results[b]["out"] for b in range(8)], axis=1)
